# revision 1
# baseline (speedup 1.0000x reference)
"""GQA kernel for Trainium2, 8 NeuronCores.

Problem: nn_GroupQueryAttention — B=4, S=2048, E=2048, 16 heads / 4 groups,
d_head=128.  out = softmax((x@Wq) (x@Wk)^T / sqrt(d)) (x@Wv) @ Wo + biases.

Sharding: core c -> (batch b = c//2, half = c%2).  Each core handles one
batch and 2 of the 4 KV groups (= 8 of the 16 heads): Wq columns / Wo rows
split by head, Wk/Wv columns split by group.  Each core produces a partial
output projection for its batch; the host sums the two halves (the Wo
contraction is split across the pair) and adds nothing else (bo is folded
into half 0).

Device-side layout trick: all inputs are fed pre-transposed (x^T [E,S]) so
every stage's matmuls consume/produce "transposed" activations:
  qh^T[d,s] = Wq_h^T x^T        kh^T[d,t] = Wk_g^T x^T     vh[t,d] = (x^T)^T Wv
  s^T[t,q]  = kh^T.T·qh^T       e^T = exp(s^T/sqrt(d))
  ctxu^T[d,q] = vh.T·e^T        rsum[*,q] = ones.T·e^T   (replicated rows)
  ctx^T = ctxu^T * (1/rsum)     out[s,e] = ctx^T.T·Wo + bo
Softmax skips the max-subtraction: scores are ~N(0,1) (inputs are randn,
weights scaled 1/sqrt(E)), far from fp32 exp overflow.

All matmuls run in float32r (1 cycle/row at N>=256 vs 4 for fp32;
rms rel err ~1.5e-4 measured on hardware).
"""

import sys

sys.path.insert(0, "/opt/trn_rl_repo")

import numpy as np

B, S, E = 4, 2048, 2048
D = 128            # head dim
HPC = 8            # heads per core
GPC = 2            # groups per core
QC = HPC * D       # 1024 Wq cols per core
KV = GPC * D       # 256 Wk/Wv cols per core
NE = E // D        # 16 contraction chunks
ST = S // 512      # 4 tiles of 512 along s/t
NT = S // D        # 16 t-chunks of 128
N_CORES = 8

_PROGRAM = None


def _build():
    from contextlib import ExitStack

    import concourse.bass as bass
    import concourse.mybir as mybir
    import concourse.tile as tile
    from concourse import bacc

    F32 = mybir.dt.float32
    F32R = mybir.dt.float32r
    Exp = mybir.ActivationFunctionType.Exp
    SCALE = 1.0 / float(np.sqrt(D))

    nc = bacc.Bacc("TRN2", target_bir_lowering=False, debug=False)
    xq = nc.dram_tensor("xq", [E, S], F32R, kind="ExternalInput")
    xk = nc.dram_tensor("xk", [E, S], F32R, kind="ExternalInput")
    xv = nc.dram_tensor("xv", [E, S], F32R, kind="ExternalInput")
    wq = nc.dram_tensor("wq", [E, QC], F32R, kind="ExternalInput")
    wk = nc.dram_tensor("wk", [E, KV], F32R, kind="ExternalInput")
    wv = nc.dram_tensor("wv", [E, KV], F32R, kind="ExternalInput")
    wo = nc.dram_tensor("wo", [QC, E], F32R, kind="ExternalInput")
    bq = nc.dram_tensor("bq", [QC], F32, kind="ExternalInput")
    bk = nc.dram_tensor("bk", [KV], F32, kind="ExternalInput")
    bv = nc.dram_tensor("bv", [KV], F32, kind="ExternalInput")
    bo = nc.dram_tensor("bo", [E], F32, kind="ExternalInput")
    out = nc.dram_tensor("out_p", [S, E], F32, kind="ExternalOutput")

    def bcast(dram, n):
        return bass.AP(tensor=dram.ap().tensor, offset=0, ap=[[0, D], [1, n]])

    with tile.TileContext(nc) as tc:
        with ExitStack() as top:
            const = top.enter_context(tc.tile_pool(name="const", bufs=1))
            acts = top.enter_context(tc.tile_pool(name="acts", bufs=1))

            bq_sb = const.tile([D, HPC], F32)
            nc.sync.dma_start(out=bq_sb, in_=bq.ap().rearrange("(h d) -> d h", d=D))
            bk_sb = const.tile([D, GPC], F32)
            nc.sync.dma_start(out=bk_sb, in_=bk.ap().rearrange("(g d) -> d g", d=D))
            bv_rep = const.tile([D, KV], F32)
            nc.sync.dma_start(out=bv_rep, in_=bcast(bv, KV))
            bo_rep = const.tile([D, E], F32)
            nc.sync.dma_start(out=bo_rep, in_=bcast(bo, E))
            ones_f = const.tile([D, D], F32)
            nc.vector.memset(ones_f, 1.0)
            ones_sb = const.tile([D, D], F32R)
            nc.vector.tensor_copy(out=ones_sb, in_=ones_f)

            # persistent activations: k^T per group, vh per t-subtile,
            # qh^T / ctx^T share one 9-slot group (qh[h] dies as cx[h] is born)
            kT = [acts.tile([D, S], F32R, name=f"kT{g}") for g in range(GPC)]
            vh = [acts.tile([D, KV], F32R, name=f"vh{t}") for t in range(NT)]

            def qcx_tile(name):
                return acts.tile([D, S], F32R, name=name, tag="qcx", bufs=9)

            # ---- Phase A: K/V projections (both groups) ----
            with (
                tc.tile_pool(name="pa", bufs=2) as pa,
                tc.tile_pool(name="psa", bufs=2, space="PSUM") as psa,
            ):
                wk_sb = pa.tile([D, NE, KV], F32R, bufs=1)
                nc.sync.dma_start(
                    out=wk_sb, in_=wk.ap().rearrange("(n p) c -> p n c", p=D)
                )
                wv_sb = pa.tile([D, NE, KV], F32R, bufs=1)
                nc.sync.dma_start(
                    out=wv_sb, in_=wv.ap().rearrange("(n p) c -> p n c", p=D)
                )
                for tt in range(ST):
                    ps_k = [
                        psa.tile([D, 512], F32, name=f"ps_k{g}_{tt}", tag=f"psk{g}")
                        for g in range(GPC)
                    ]
                    for e in range(NE):
                        xk_ch = pa.tile(
                            [D, 512], F32R, name=f"xk_{tt}_{e}", tag="xk_ch", bufs=3
                        )
                        nc.sync.dma_start(
                            out=xk_ch,
                            in_=xk.ap()[e * D : (e + 1) * D, tt * 512 : (tt + 1) * 512],
                        )
                        for g in range(GPC):
                            nc.tensor.matmul(
                                ps_k[g],
                                wk_sb[:, e, g * D : (g + 1) * D],
                                xk_ch,
                                start=(e == 0),
                                stop=(e == NE - 1),
                            )
                    for g in range(GPC):
                        nc.vector.tensor_scalar_add(
                            out=kT[g][:, tt * 512 : (tt + 1) * 512],
                            in0=ps_k[g],
                            scalar1=bk_sb[:, g : g + 1],
                        )
                for t in range(NT):
                    ps_v = psa.tile([D, KV], F32, name=f"ps_v{t}", tag="psv")
                    for e in range(NE):
                        xv_ch = pa.tile(
                            [D, D], F32R, name=f"xv_{t}_{e}", tag="xv_ch", bufs=4
                        )
                        nc.sync.dma_start(
                            out=xv_ch,
                            in_=xv.ap()[e * D : (e + 1) * D, t * D : (t + 1) * D],
                        )
                        nc.tensor.matmul(
                            ps_v,
                            xv_ch,
                            wv_sb[:, e, :],
                            start=(e == 0),
                            stop=(e == NE - 1),
                        )
                    nc.vector.tensor_add(out=vh[t], in0=ps_v, in1=bv_rep)

            # ---- Phase B: Q projection (all heads), q^T layout ----
            qh = []
            with (
                tc.tile_pool(name="pb", bufs=2) as pb,
                tc.tile_pool(name="psb", bufs=2, space="PSUM") as psb,
            ):
                wq_r = wq.ap().rearrange("(n p) c -> p n c", p=D)  # [128,16,1024]
                for st in range(ST):
                    xq_ch = []
                    for e in range(NE):
                        t_ = pb.tile(
                            [D, 512], F32R, name=f"xq_{st}_{e}", tag="xq_ch", bufs=18
                        )
                        nc.sync.dma_start(
                            out=t_,
                            in_=xq.ap()[e * D : (e + 1) * D, st * 512 : (st + 1) * 512],
                        )
                        xq_ch.append(t_)
                    for h in range(HPC):
                        wq_h = pb.tile(
                            [D, NE, D], F32R, name=f"wq_{st}_{h}", tag="wq_h", bufs=2
                        )
                        nc.sync.dma_start(
                            out=wq_h, in_=wq_r[:, :, h * D : (h + 1) * D]
                        )
                        ps_q = psb.tile([D, 512], F32, name=f"ps_q{st}_{h}", tag="psq")
                        for e in range(NE):
                            nc.tensor.matmul(
                                ps_q,
                                wq_h[:, e, :],
                                xq_ch[e],
                                start=(e == 0),
                                stop=(e == NE - 1),
                            )
                        if st == 0:
                            qh.append(qcx_tile(f"qh{h}"))
                        nc.vector.tensor_scalar_add(
                            out=qh[h][:, st * 512 : (st + 1) * 512],
                            in0=ps_q,
                            scalar1=bq_sb[:, h : h + 1],
                        )

            # ---- Phase C: attention per head ----
            cx = []
            with (
                tc.tile_pool(name="pc", bufs=2) as pc,
                tc.tile_pool(name="psc", bufs=2, space="PSUM") as psc,
            ):
                for h in range(HPC):
                    g = h // (HPC // GPC)
                    cxt = qcx_tile(f"cx{h}")
                    cx.append(cxt)
                    for qt in range(ST):
                        qsl = qh[h][:, qt * 512 : (qt + 1) * 512]
                        ps_ctx = psc.tile(
                            [D, 512], F32, name=f"ps_ctx{h}_{qt}", tag="ps_ctx"
                        )
                        ps_rs = psc.tile(
                            [D, 512], F32, name=f"ps_rs{h}_{qt}", tag="ps_rs"
                        )
                        for tp in range(NT // 2):
                            ps_sT = psc.tile(
                                [D, 2, 512], F32, name=f"ps_sT{h}_{qt}_{tp}", tag="ps_sT"
                            )
                            for j in range(2):
                                t_c = tp * 2 + j
                                nc.tensor.matmul(
                                    ps_sT[:, j, :],
                                    kT[g][:, t_c * D : (t_c + 1) * D],
                                    qsl,
                                    start=True,
                                    stop=True,
                                )
                            ex = pc.tile(
                                [D, 2, 512], F32R, name=f"ex{h}_{qt}_{tp}",
                                tag="ex", bufs=3,
                            )
                            nc.scalar.activation(out=ex, in_=ps_sT, func=Exp, scale=SCALE)
                            for j in range(2):
                                t_c = tp * 2 + j
                                nc.tensor.matmul(
                                    ps_ctx,
                                    vh[t_c][:, g * D : (g + 1) * D],
                                    ex[:, j, :],
                                    start=(t_c == 0),
                                    stop=(t_c == NT - 1),
                                )
                                nc.tensor.matmul(
                                    ps_rs,
                                    ones_sb,
                                    ex[:, j, :],
                                    start=(t_c == 0),
                                    stop=(t_c == NT - 1),
                                )
                        rr = pc.tile([D, 512], F32, name=f"rr{h}_{qt}", tag="rr", bufs=2)
                        nc.vector.reciprocal(out=rr, in_=ps_rs)
                        nc.vector.tensor_mul(
                            out=cxt[:, qt * 512 : (qt + 1) * 512], in0=ps_ctx, in1=rr
                        )

            # ---- Phase D: output projection + bias ----
            with (
                tc.tile_pool(name="pd", bufs=2) as pd,
                tc.tile_pool(name="psd", bufs=3, space="PSUM") as psd,
            ):
                wo_r = wo.ap().rearrange("(c p) e -> p c e", p=D)  # [128,8,2048]
                for et in range(ST):
                    wo_t = pd.tile(
                        [D, HPC, 512], F32R, name=f"wo_{et}", tag="wo_t", bufs=2
                    )
                    nc.sync.dma_start(out=wo_t, in_=wo_r[:, :, et * 512 : (et + 1) * 512])
                    for ss in range(NT):
                        ps_o = psd.tile([D, 512], F32, name=f"ps_o{et}_{ss}", tag="ps_o")
                        for hh in range(HPC):
                            nc.tensor.matmul(
                                ps_o,
                                cx[hh][:, ss * D : (ss + 1) * D],
                                wo_t[:, hh, :],
                                start=(hh == 0),
                                stop=(hh == HPC - 1),
                            )
                        ot = pd.tile([D, 512], F32, name=f"ot{et}_{ss}", tag="ot", bufs=3)
                        nc.vector.tensor_add(
                            out=ot, in0=ps_o, in1=bo_rep[:, et * 512 : (et + 1) * 512]
                        )
                        nc.sync.dma_start(
                            out=out.ap()[ss * D : (ss + 1) * D, et * 512 : (et + 1) * 512],
                            in_=ot,
                        )

    nc.compile()
    return nc


def _get_program():
    global _PROGRAM
    if _PROGRAM is None:
        _PROGRAM = _build()
    return _PROGRAM


def make_in_maps(q, k, v, Wq, bq, Wk, bk, Wv, bv, Wo, bo):
    f32 = lambda a: np.asarray(a, dtype=np.float32)
    q, k, v = f32(q), f32(k), f32(v)
    Wq, bq, Wk, bk, Wv, bv, Wo, bo = (
        f32(Wq), f32(bq), f32(Wk), f32(bk), f32(Wv), f32(bv), f32(Wo), f32(bo)
    )
    in_maps = []
    for c in range(N_CORES):
        b, half = c // 2, c % 2
        in_maps.append(
            {
                "xq": np.ascontiguousarray(q[b].T),
                "xk": np.ascontiguousarray(k[b].T),
                "xv": np.ascontiguousarray(v[b].T),
                "wq": np.ascontiguousarray(Wq[:, half * QC : (half + 1) * QC]),
                "wk": np.ascontiguousarray(Wk[:, half * KV : (half + 1) * KV]),
                "wv": np.ascontiguousarray(Wv[:, half * KV : (half + 1) * KV]),
                "wo": np.ascontiguousarray(Wo[half * QC : (half + 1) * QC, :]),
                "bq": np.ascontiguousarray(bq[half * QC : (half + 1) * QC]),
                "bk": np.ascontiguousarray(bk[half * KV : (half + 1) * KV]),
                "bv": np.ascontiguousarray(bv[half * KV : (half + 1) * KV]),
                "bo": bo if half == 0 else np.zeros_like(bo),
            }
        )
    return in_maps


def combine_results(results):
    out = np.empty((B, S, E), np.float32)
    for b in range(B):
        out[b] = np.asarray(results[2 * b]["out_p"]) + np.asarray(
            results[2 * b + 1]["out_p"]
        )
    return out


def kernel(q, k, v, Wq, bq, Wk, bk, Wv, bv, Wo, bo):
    from concourse.bass_utils import run_bass_kernel_spmd

    nc = _get_program()
    in_maps = make_in_maps(q, k, v, Wq, bq, Wk, bk, Wv, bv, Wo, bo)
    res = run_bass_kernel_spmd(nc, in_maps, core_ids=list(range(N_CORES)))
    return combine_results(res.results)



# revision 2
# speedup vs baseline: 1.1176x; 1.1176x over previous
"""GQA kernel for Trainium2, 8 NeuronCores.

Problem: nn_GroupQueryAttention — B=4, S=2048, E=2048, 16 heads / 4 groups,
d_head=128.  out = softmax((x@Wq) (x@Wk)^T / sqrt(d)) (x@Wv) @ Wo + biases.

Sharding: core c -> (batch b = c//2, half = c%2).  Each core handles one
batch and 2 of the 4 KV groups (= 8 of the 16 heads): Wq columns / Wo rows
split by head, Wk/Wv columns split by group.  Each core produces a partial
output projection for its batch; the host sums the two halves (bo is folded
into half 0).

Layout: inputs are fed pre-transposed (x^T [E,S]) in bf16; all projection
weights are bf16 (the PE runs bf16 and f32r both at 1 cycle/row, bf16 halves
the start-critical HBM traffic).  Intermediate activations stay f32r:
  qh^T[d,s] = Wq_h^T x^T      kT[d,t] = Wk_g^T x^T      vT[d,t] = Wv_g^T x^T
  vh[t,d]   = PE-transpose of vT (so attn·V gets a [t,d] stationary)
  s^T[t,q]  = kT.T·qh^T       e^T = exp(s^T/sqrt(d))
  ctxu^T[d,q] = vh.T·e^T      rsum[*,q] = ones.T·(e^T pair sums)
  ctx^T = ctxu^T * (1/rsum)   out[s,e] = ctx^T.T·Wo + bo
Softmax skips the max-subtraction: scores are ~N(0,1), far from fp32 exp
overflow.

Perf structure: phase C is software-pipelined (scores of step tp issue
before attn·V of tp-1, so the PE never waits on the scalar engine's exp);
exp tiles are pair-summed on DVE/GpSimd (alternating) so the rowsum
ones-matmul streams half the rows; wq/wo are resident via single contiguous
DMAs issued on a second queue ahead of use.
"""

import sys

sys.path.insert(0, "/opt/trn_rl_repo")

import ml_dtypes
import numpy as np

B, S, E = 4, 2048, 2048
D = 128            # head dim
HPC = 8            # heads per core
GPC = 2            # groups per core
QC = HPC * D       # 1024 Wq cols per core
KV = GPC * D       # 256 Wk/Wv cols per core
NE = E // D        # 16 contraction chunks
ST = S // 512      # 4 tiles of 512 along s/t
NT = S // D        # 16 t-chunks of 128
N_CORES = 8

_PROGRAM = None


def _build():
    from contextlib import ExitStack

    import concourse.bass as bass
    import concourse.mybir as mybir
    import concourse.tile as tile
    from concourse import bacc

    F32 = mybir.dt.float32
    F32R = mybir.dt.float32r
    BF16 = mybir.dt.bfloat16
    Exp = mybir.ActivationFunctionType.Exp
    SCALE = 1.0 / float(np.sqrt(D))

    nc = bacc.Bacc("TRN2", target_bir_lowering=False, debug=False)
    xq = nc.dram_tensor("xq", [E, S], BF16, kind="ExternalInput")
    xk = nc.dram_tensor("xk", [E, S], BF16, kind="ExternalInput")
    xv = nc.dram_tensor("xv", [E, S], BF16, kind="ExternalInput")
    wq = nc.dram_tensor("wq", [E, QC], BF16, kind="ExternalInput")
    wk = nc.dram_tensor("wk", [E, KV], BF16, kind="ExternalInput")
    wv = nc.dram_tensor("wv", [E, KV], BF16, kind="ExternalInput")
    wo = nc.dram_tensor("wo", [QC, E], F32R, kind="ExternalInput")
    bq = nc.dram_tensor("bq", [QC], F32, kind="ExternalInput")
    bk = nc.dram_tensor("bk", [KV], F32, kind="ExternalInput")
    bv = nc.dram_tensor("bv", [KV], F32, kind="ExternalInput")
    bo = nc.dram_tensor("bo", [E], F32, kind="ExternalInput")
    ident = nc.dram_tensor("ident", [D, D], F32R, kind="ExternalInput")
    onesd = nc.dram_tensor("onesd", [D, D], F32R, kind="ExternalInput")
    out = nc.dram_tensor("out_p", [S, E], F32, kind="ExternalOutput")

    def bcast(dram, n):
        return bass.AP(tensor=dram.ap().tensor, offset=0, ap=[[0, D], [1, n]])

    with tile.TileContext(nc) as tc:
        with ExitStack() as top:
            const = top.enter_context(tc.tile_pool(name="const", bufs=1))
            acts = top.enter_context(tc.tile_pool(name="acts", bufs=1))

            bq_sb = const.tile([D, HPC], F32)
            nc.sync.dma_start(out=bq_sb, in_=bq.ap().rearrange("(h d) -> d h", d=D))
            bk_sb = const.tile([D, GPC], F32)
            nc.sync.dma_start(out=bk_sb, in_=bk.ap().rearrange("(g d) -> d g", d=D))
            bv_sb = const.tile([D, GPC], F32)
            nc.sync.dma_start(out=bv_sb, in_=bv.ap().rearrange("(g d) -> d g", d=D))
            ones_sb = const.tile([D, D], F32R)
            nc.sync.dma_start(out=ones_sb, in_=onesd.ap())
            id_sb = const.tile([D, D], F32R)
            nc.sync.dma_start(out=id_sb, in_=ident.ap())

            # persistent activations: k^T per group, vh per t-subtile,
            # qh^T / ctx^T share one 9-slot group (qh[h] dies as cx[h] is born)
            kT = [acts.tile([D, S], F32R, name=f"kT{g}") for g in range(GPC)]
            vh = [acts.tile([D, KV], F32R, name=f"vh{t}") for t in range(NT)]

            def qcx_tile(name):
                return acts.tile([D, S], F32R, name=name, tag="qcx", bufs=9)

            # ---- Phases A (K/V proj) + B (Q proj), pools coexist so the
            # scheduler can overlap B's start with A's DMA-bound tail ----
            qh = []
            with ExitStack() as ab:
                pa = ab.enter_context(tc.tile_pool(name="pa", bufs=2))
                psa = ab.enter_context(tc.tile_pool(name="psa", bufs=1, space="PSUM"))
                pb = ab.enter_context(tc.tile_pool(name="pb", bufs=2))
                psb = ab.enter_context(tc.tile_pool(name="psb", bufs=2, space="PSUM"))

                # B-phase weights: one contiguous DMA on the scalar-engine
                # queue — runs concurrently with phase A's sync-queue stream.
                wq_sb = pb.tile([D, NE, QC], BF16, bufs=1)
                nc.scalar.dma_start(
                    out=wq_sb, in_=wq.ap().rearrange("(n p) c -> p n c", p=D)
                )

                wk_sb = pa.tile([D, NE, KV], BF16, bufs=1)
                nc.sync.dma_start(
                    out=wk_sb, in_=wk.ap().rearrange("(n p) c -> p n c", p=D)
                )
                wv_sb = pa.tile([D, NE, KV], BF16, bufs=1)
                nc.sync.dma_start(
                    out=wv_sb, in_=wv.ap().rearrange("(n p) c -> p n c", p=D)
                )

                def emit_transposes(tt, vts):
                    for g in range(GPC):
                        for j in range(ST):
                            t_c = tt * ST + j
                            ps_tr = psa.tile(
                                [D, D], F32R, name=f"ps_tr{g}_{tt}_{j}",
                                tag="pstr", bufs=2,
                            )
                            nc.tensor.transpose(
                                ps_tr, vts[g][:, j * D : (j + 1) * D], id_sb
                            )
                            nc.vector.tensor_copy(
                                out=vh[t_c][:, g * D : (g + 1) * D], in_=ps_tr
                            )

                vT_prev = None
                for tt in range(ST):
                    ps_k = [
                        psa.tile([D, 512], F32, name=f"ps_k{g}_{tt}", tag=f"psk{g}")
                        for g in range(GPC)
                    ]
                    ps_vT = [
                        psa.tile([D, 512], F32, name=f"ps_vT{g}_{tt}", tag=f"psv{g}")
                        for g in range(GPC)
                    ]
                    for e in range(NE):
                        xk_ch = pa.tile(
                            [D, 512], BF16, name=f"xk_{tt}_{e}", tag="xk_ch", bufs=4
                        )
                        nc.sync.dma_start(
                            out=xk_ch,
                            in_=xk.ap()[e * D : (e + 1) * D, tt * 512 : (tt + 1) * 512],
                        )
                        xv_ch = pa.tile(
                            [D, 512], BF16, name=f"xv_{tt}_{e}", tag="xv_ch", bufs=4
                        )
                        nc.sync.dma_start(
                            out=xv_ch,
                            in_=xv.ap()[e * D : (e + 1) * D, tt * 512 : (tt + 1) * 512],
                        )
                        for g in range(GPC):
                            nc.tensor.matmul(
                                ps_k[g],
                                wk_sb[:, e, g * D : (g + 1) * D],
                                xk_ch,
                                start=(e == 0),
                                stop=(e == NE - 1),
                            )
                        for g in range(GPC):
                            nc.tensor.matmul(
                                ps_vT[g],
                                wv_sb[:, e, g * D : (g + 1) * D],
                                xv_ch,
                                start=(e == 0),
                                stop=(e == NE - 1),
                            )
                    vT_cur = []
                    for g in range(GPC):
                        nc.vector.tensor_scalar_add(
                            out=kT[g][:, tt * 512 : (tt + 1) * 512],
                            in0=ps_k[g],
                            scalar1=bk_sb[:, g : g + 1],
                        )
                        vts = pa.tile(
                            [D, 512], F32R, name=f"vts{g}_{tt}", tag=f"vts{g}", bufs=2
                        )
                        nc.vector.tensor_scalar_add(
                            out=vts, in0=ps_vT[g], scalar1=bv_sb[:, g : g + 1]
                        )
                        vT_cur.append(vts)
                    # transposes for the previous tt (pipelined so the PE's
                    # transpose never waits on this tt's DVE drain)
                    if vT_prev is not None:
                        emit_transposes(tt - 1, vT_prev)
                    vT_prev = vT_cur
                emit_transposes(ST - 1, vT_prev)

                # ---- Phase B: Q projection, wq resident, h-outer ----
                for st in range(ST):
                    xq_ch = []
                    for e in range(NE):
                        t_ = pb.tile(
                            [D, 512], BF16, name=f"xq_{st}_{e}", tag="xq_ch", bufs=24
                        )
                        nc.scalar.dma_start(
                            out=t_,
                            in_=xq.ap()[e * D : (e + 1) * D, st * 512 : (st + 1) * 512],
                        )
                        xq_ch.append(t_)
                    for h in range(HPC):
                        ps_q = psb.tile(
                            [D, 512], F32, name=f"ps_q{st}_{h}", tag="psq", bufs=2
                        )
                        for e in range(NE):
                            nc.tensor.matmul(
                                ps_q,
                                wq_sb[:, e, h * D : (h + 1) * D],
                                xq_ch[e],
                                start=(e == 0),
                                stop=(e == NE - 1),
                            )
                        if st == 0:
                            qh.append(qcx_tile(f"qh{h}"))
                        nc.vector.tensor_scalar_add(
                            out=qh[h][:, st * 512 : (st + 1) * 512],
                            in0=ps_q,
                            scalar1=bq_sb[:, h : h + 1],
                        )

            # ---- Phases C (attention) + D (output projection) ----
            cx = []
            with ExitStack() as cd:
                pw = cd.enter_context(tc.tile_pool(name="pw", bufs=1))
                # prefetch D-phase weights during C (sync queue idle in C)
                wo_sb = pw.tile([D, HPC, E], F32R)
                nc.sync.dma_start(
                    out=wo_sb, in_=wo.ap().rearrange("(c p) e -> p c e", p=D)
                )
                bo_rep = pw.tile([D, E], F32)
                nc.scalar.dma_start(out=bo_rep, in_=bcast(bo, E))

                with ExitStack() as cc:
                    pc = cc.enter_context(tc.tile_pool(name="pc", bufs=2))
                    psc = cc.enter_context(
                        tc.tile_pool(name="psc", bufs=2, space="PSUM")
                    )
                    for h in range(HPC):
                        g = h // (HPC // GPC)
                        cxt = qcx_tile(f"cx{h}")
                        cx.append(cxt)
                        for qt in range(ST):
                            qsl = qh[h][:, qt * 512 : (qt + 1) * 512]
                            ps_ctx = psc.tile(
                                [D, 512], F32, name=f"ps_ctx{h}_{qt}", tag="ps_ctx"
                            )
                            ps_rs = psc.tile(
                                [D, 512], F32, name=f"ps_rs{h}_{qt}", tag="ps_rs"
                            )

                            def emit_av(tp, ex, exs):
                                for j in range(2):
                                    t_c = tp * 2 + j
                                    nc.tensor.matmul(
                                        ps_ctx,
                                        vh[t_c][:, g * D : (g + 1) * D],
                                        ex[:, j, :],
                                        start=(t_c == 0),
                                        stop=(t_c == NT - 1),
                                    )
                                nc.tensor.matmul(
                                    ps_rs,
                                    ones_sb,
                                    exs,
                                    start=(tp == 0),
                                    stop=(tp == NT // 2 - 1),
                                )

                            prev = None
                            for tp in range(NT // 2):
                                ps_sT = psc.tile(
                                    [D, 2, 512], F32, name=f"ps_sT{h}_{qt}_{tp}",
                                    tag="ps_sT",
                                )
                                for j in range(2):
                                    t_c = tp * 2 + j
                                    nc.tensor.matmul(
                                        ps_sT[:, j, :],
                                        kT[g][:, t_c * D : (t_c + 1) * D],
                                        qsl,
                                        start=True,
                                        stop=True,
                                    )
                                ex = pc.tile(
                                    [D, 2, 512], F32R, name=f"ex{h}_{qt}_{tp}",
                                    tag="ex", bufs=3,
                                )
                                nc.scalar.activation(
                                    out=ex, in_=ps_sT, func=Exp, scale=SCALE
                                )
                                exs = pc.tile(
                                    [D, 512], F32R, name=f"exs{h}_{qt}_{tp}",
                                    tag="exs", bufs=3,
                                )
                                eng = nc.vector if tp % 2 == 0 else nc.gpsimd
                                eng.tensor_add(
                                    out=exs, in0=ex[:, 0, :], in1=ex[:, 1, :]
                                )
                                # attn·V + rowsum for the PREVIOUS tp: the PE
                                # issues this tp's scores first so it never
                                # stalls on the scalar engine's exp
                                if prev is not None:
                                    emit_av(*prev)
                                prev = (tp, ex, exs)
                            emit_av(*prev)

                            rr = pc.tile(
                                [D, 512], F32, name=f"rr{h}_{qt}", tag="rr", bufs=2
                            )
                            nc.vector.reciprocal(out=rr, in_=ps_rs)
                            nc.vector.tensor_mul(
                                out=cxt[:, qt * 512 : (qt + 1) * 512],
                                in0=ps_ctx,
                                in1=rr,
                            )

                # ---- Phase D: output projection + bias ----
                with ExitStack() as dd:
                    pd = dd.enter_context(tc.tile_pool(name="pd", bufs=2))
                    psd = dd.enter_context(
                        tc.tile_pool(name="psd", bufs=3, space="PSUM")
                    )
                    for et in range(ST):
                        for ss in range(NT):
                            ps_o = psd.tile(
                                [D, 512], F32, name=f"ps_o{et}_{ss}", tag="ps_o"
                            )
                            for hh in range(HPC):
                                nc.tensor.matmul(
                                    ps_o,
                                    cx[hh][:, ss * D : (ss + 1) * D],
                                    wo_sb[:, hh, et * 512 : (et + 1) * 512],
                                    start=(hh == 0),
                                    stop=(hh == HPC - 1),
                                )
                            ot = pd.tile(
                                [D, 512], F32, name=f"ot{et}_{ss}", tag="ot", bufs=3
                            )
                            nc.vector.tensor_add(
                                out=ot,
                                in0=ps_o,
                                in1=bo_rep[:, et * 512 : (et + 1) * 512],
                            )
                            nc.sync.dma_start(
                                out=out.ap()[
                                    ss * D : (ss + 1) * D, et * 512 : (et + 1) * 512
                                ],
                                in_=ot,
                            )

    nc.compile()
    return nc


def _get_program():
    global _PROGRAM
    if _PROGRAM is None:
        _PROGRAM = _build()
    return _PROGRAM


def make_in_maps(q, k, v, Wq, bq, Wk, bk, Wv, bv, Wo, bo):
    f32 = lambda a: np.asarray(a, dtype=np.float32)
    bf16 = lambda a: np.ascontiguousarray(a).astype(ml_dtypes.bfloat16)
    q, k, v = f32(q), f32(k), f32(v)
    Wq, bq, Wk, bk, Wv, bv, Wo, bo = (
        f32(Wq), f32(bq), f32(Wk), f32(bk), f32(Wv), f32(bv), f32(Wo), f32(bo)
    )
    ident = np.eye(D, dtype=np.float32)
    onesd = np.ones((D, D), dtype=np.float32)
    in_maps = []
    for c in range(N_CORES):
        b, half = c // 2, c % 2
        in_maps.append(
            {
                "xq": bf16(q[b].T),
                "xk": bf16(k[b].T),
                "xv": bf16(v[b].T),
                "wq": bf16(Wq[:, half * QC : (half + 1) * QC]),
                "wk": bf16(Wk[:, half * KV : (half + 1) * KV]),
                "wv": bf16(Wv[:, half * KV : (half + 1) * KV]),
                "wo": np.ascontiguousarray(Wo[half * QC : (half + 1) * QC, :]),
                "bq": np.ascontiguousarray(bq[half * QC : (half + 1) * QC]),
                "bk": np.ascontiguousarray(bk[half * KV : (half + 1) * KV]),
                "bv": np.ascontiguousarray(bv[half * KV : (half + 1) * KV]),
                "bo": bo if half == 0 else np.zeros_like(bo),
                "ident": ident,
                "onesd": onesd,
            }
        )
    return in_maps


def combine_results(results):
    out = np.empty((B, S, E), np.float32)
    for b in range(B):
        out[b] = np.asarray(results[2 * b]["out_p"]) + np.asarray(
            results[2 * b + 1]["out_p"]
        )
    return out


def kernel(q, k, v, Wq, bq, Wk, bk, Wv, bv, Wo, bo):
    from concourse.bass_utils import run_bass_kernel_spmd

    nc = _get_program()
    in_maps = make_in_maps(q, k, v, Wq, bq, Wk, bk, Wv, bv, Wo, bo)
    res = run_bass_kernel_spmd(nc, in_maps, core_ids=list(range(N_CORES)))
    return combine_results(res.results)


# revision 8
# speedup vs baseline: 1.5680x; 1.4030x over previous
"""GQA kernel for Trainium2, 8 NeuronCores.

Problem: nn_GroupQueryAttention — B=4, S=2048, E=2048, 16 heads / 4 groups,
d_head=128.  out = softmax((x@Wq) (x@Wk)^T / sqrt(d)) (x@Wv) @ Wo + biases.

Sharding: core c -> (batch b = c//2, half = c%2).  Each core handles one
batch and 2 of the 4 KV groups (= 8 of the 16 heads): Wq columns / Wo rows
split by head, Wk/Wv columns split by group.  Each core produces a partial
output projection for its batch; the host sums the two halves (bo is folded
into half 0).

Layout: inputs are fed pre-transposed (x^T [E,S]) in bf16; projection
weights are bf16, host-packed partition-major so each SBUF load is one DMA
with multi-KB descriptors.  Intermediate activations stay f32r:
  qh^T[d,s] = Wq_h^T x^T      kT[d,t] = Wk_g^T x^T      vT[d,t] = Wv_g^T x^T
  vh[t,d]   = PE-transpose of vT (so attn·V gets a [t,d] stationary)
  s^T[t,q]  = kT.T·qh^T       e^T = exp(s^T/sqrt(d))
  ctxu^T[d,q] = vh.T·e^T      rsum[*,q] = ones.T·e^T  (replicated rows)
  ctx^T = ctxu^T * (1/rsum)   out[s,e] = ctx^T.T·Wo + bo
Softmax skips the max-subtraction: scores are ~N(0,1), far from fp32 exp
overflow.

Perf structure: phase C is a single flat software pipeline across every
(head, q-tile, t-pair) step — the attn·V + rowsum matmuls for step i are
emitted after the score matmuls of step i+1, so the PE stream never drains
waiting on the scalar engine's exp (and stays at full p-state clock).
"""

import sys

sys.path.insert(0, "/opt/trn_rl_repo")

import ml_dtypes
import numpy as np

B, S, E = 4, 2048, 2048
D = 128            # head dim
HPC = 8            # heads per core
GPC = 2            # groups per core
QC = HPC * D       # 1024 Wq cols per core
KV = GPC * D       # 256 Wk/Wv cols per core
NE = E // D        # 16 contraction chunks
ST = S // 512      # 4 tiles of 512 along s/t
NT = S // D        # 16 t-chunks of 128
N_CORES = 8

# consts_p column offsets: bq [0:8], bk [8:10], bv [10:12],
# ones [12:140], ident [140:268]
CST_W = 268

_PROGRAM = None


def _build():
    from contextlib import ExitStack

    import concourse.bass as bass
    import concourse.mybir as mybir
    import concourse.tile as tile
    from concourse import bacc

    F32 = mybir.dt.float32
    F32R = mybir.dt.float32r
    BF16 = mybir.dt.bfloat16
    Exp = mybir.ActivationFunctionType.Exp
    SCALE = 1.0 / float(np.sqrt(D))

    nc = bacc.Bacc("TRN2", target_bir_lowering=False, debug=False)
    xq = nc.dram_tensor("xq", [E, S], BF16, kind="ExternalInput")
    xk = nc.dram_tensor("xk", [E, S], BF16, kind="ExternalInput")
    xv = nc.dram_tensor("xv", [E, S], BF16, kind="ExternalInput")
    # host-packed, partition-major: one fat-descriptor DMA each
    wkv = nc.dram_tensor("wkv", [D, NE * 2 * KV], BF16, kind="ExternalInput")
    wqp = nc.dram_tensor("wqp", [D, NE * QC], BF16, kind="ExternalInput")
    wop = nc.dram_tensor("wop", [D, HPC * E], F32R, kind="ExternalInput")
    cstb = nc.dram_tensor("cstb", [D, 12], F32, kind="ExternalInput")
    cstm = nc.dram_tensor("cstm", [D, 2 * D], F32R, kind="ExternalInput")
    bo = nc.dram_tensor("bo", [E], F32, kind="ExternalInput")
    out = nc.dram_tensor("out_p", [S, E], F32, kind="ExternalOutput")

    def bcast(dram, n):
        return bass.AP(tensor=dram.ap().tensor, offset=0, ap=[[0, D], [1, n]])

    with tile.TileContext(nc) as tc:
        with ExitStack() as top:
            const = top.enter_context(tc.tile_pool(name="const", bufs=1))
            acts = top.enter_context(tc.tile_pool(name="acts", bufs=1))

            cstb_sb = const.tile([D, 12], F32)
            nc.sync.dma_start(out=cstb_sb, in_=cstb.ap())
            cstm_sb = const.tile([D, 2 * D], F32R)
            nc.sync.dma_start(out=cstm_sb, in_=cstm.ap())
            bq_sb = cstb_sb[:, 0:HPC]
            bk_sb = cstb_sb[:, HPC : HPC + GPC]
            bv_sb = cstb_sb[:, HPC + GPC : HPC + 2 * GPC]
            ones_sb = cstm_sb[:, 0:D]
            id_sb = cstm_sb[:, D : 2 * D]

            # persistent activations: k^T per group, vh per t-subtile,
            # qh^T / ctx^T share one 9-slot group (qh[h] dies as cx[h] is born)
            kT = [acts.tile([D, S], F32R, name=f"kT{g}") for g in range(GPC)]
            vh = [acts.tile([D, KV], F32R, name=f"vh{t}") for t in range(NT)]

            def qcx_tile(name):
                return acts.tile([D, S], F32R, name=name, tag="qcx", bufs=9)

            # ---- Phases A (K/V proj) + B (Q proj) ----
            qh = []
            with ExitStack() as ab:
                pa = ab.enter_context(tc.tile_pool(name="pa", bufs=2))
                psa = ab.enter_context(tc.tile_pool(name="psa", bufs=1, space="PSUM"))
                pb = ab.enter_context(tc.tile_pool(name="pb", bufs=2))
                psb = ab.enter_context(tc.tile_pool(name="psb", bufs=2, space="PSUM"))

                # B-phase weights: one fat DMA on the scalar-engine queue —
                # runs during phase A without stealing the sync queue's order.
                wq_sb = pb.tile([D, NE, QC], BF16, bufs=1)
                nc.scalar.dma_start(out=wq_sb, in_=wqp.ap().rearrange("p (n c) -> p n c", n=NE))

                wk_sb = pa.tile([D, NE, KV], BF16, bufs=1)
                wv_sb = pa.tile([D, NE, KV], BF16, bufs=1)
                kvv = wkv.ap().rearrange("p (n t c) -> p n t c", n=NE, t=2)
                nc.sync.dma_start(out=wk_sb, in_=kvv[:, :, 0, :])
                nc.sync.dma_start(out=wv_sb, in_=kvv[:, :, 1, :])

                def emit_transposes(tt, vts):
                    for g in range(GPC):
                        for j in range(ST):
                            t_c = tt * ST + j
                            ps_tr = psa.tile(
                                [D, D], F32R, name=f"ps_tr{g}_{tt}_{j}",
                                tag="pstr", bufs=2,
                            )
                            nc.tensor.transpose(
                                ps_tr, vts[g][:, j * D : (j + 1) * D], id_sb
                            )
                            nc.vector.tensor_copy(
                                out=vh[t_c][:, g * D : (g + 1) * D], in_=ps_tr
                            )

                vT_prev = None
                for tt in range(ST):
                    ps_k = [
                        psa.tile([D, 512], F32, name=f"ps_k{g}_{tt}", tag=f"psk{g}")
                        for g in range(GPC)
                    ]
                    ps_vT = [
                        psa.tile([D, 512], F32, name=f"ps_vT{g}_{tt}", tag=f"psv{g}")
                        for g in range(GPC)
                    ]
                    for e in range(NE):
                        xk_ch = pa.tile(
                            [D, 512], BF16, name=f"xk_{tt}_{e}", tag="xk_ch", bufs=4
                        )
                        nc.sync.dma_start(
                            out=xk_ch,
                            in_=xk.ap()[e * D : (e + 1) * D, tt * 512 : (tt + 1) * 512],
                        )
                        xv_ch = pa.tile(
                            [D, 512], BF16, name=f"xv_{tt}_{e}", tag="xv_ch", bufs=4
                        )
                        nc.sync.dma_start(
                            out=xv_ch,
                            in_=xv.ap()[e * D : (e + 1) * D, tt * 512 : (tt + 1) * 512],
                        )
                        for g in range(GPC):
                            nc.tensor.matmul(
                                ps_k[g],
                                wk_sb[:, e, g * D : (g + 1) * D],
                                xk_ch,
                                start=(e == 0),
                                stop=(e == NE - 1),
                            )
                        for g in range(GPC):
                            nc.tensor.matmul(
                                ps_vT[g],
                                wv_sb[:, e, g * D : (g + 1) * D],
                                xv_ch,
                                start=(e == 0),
                                stop=(e == NE - 1),
                            )
                    vT_cur = []
                    for g in range(GPC):
                        nc.vector.tensor_scalar_add(
                            out=kT[g][:, tt * 512 : (tt + 1) * 512],
                            in0=ps_k[g],
                            scalar1=bk_sb[:, g : g + 1],
                        )
                        vts = pa.tile(
                            [D, 512], F32R, name=f"vts{g}_{tt}", tag=f"vts{g}", bufs=2
                        )
                        nc.vector.tensor_scalar_add(
                            out=vts, in0=ps_vT[g], scalar1=bv_sb[:, g : g + 1]
                        )
                        vT_cur.append(vts)
                    # transposes for the previous tt (pipelined so the PE's
                    # transpose never waits on this tt's DVE drain)
                    if vT_prev is not None:
                        emit_transposes(tt - 1, vT_prev)
                    vT_prev = vT_cur
                emit_transposes(ST - 1, vT_prev)

                # ---- Phase B: Q projection, wq resident, h-outer per st ----
                for st in range(ST):
                    xq_ch = []
                    for e in range(NE):
                        t_ = pb.tile(
                            [D, 512], BF16, name=f"xq_{st}_{e}", tag="xq_ch", bufs=24
                        )
                        nc.scalar.dma_start(
                            out=t_,
                            in_=xq.ap()[e * D : (e + 1) * D, st * 512 : (st + 1) * 512],
                        )
                        xq_ch.append(t_)
                    for h in range(HPC):
                        ps_q = psb.tile(
                            [D, 512], F32, name=f"ps_q{st}_{h}", tag="psq", bufs=2
                        )
                        for e in range(NE):
                            nc.tensor.matmul(
                                ps_q,
                                wq_sb[:, e, h * D : (h + 1) * D],
                                xq_ch[e],
                                start=(e == 0),
                                stop=(e == NE - 1),
                            )
                        if st == 0:
                            qh.append(qcx_tile(f"qh{h}"))
                        nc.vector.tensor_scalar_add(
                            out=qh[h][:, st * 512 : (st + 1) * 512],
                            in0=ps_q,
                            scalar1=bq_sb[:, h : h + 1],
                        )

            # ---- Phases C (attention) + D (output projection) ----
            cx = []
            with ExitStack() as cd:
                pw = cd.enter_context(tc.tile_pool(name="pw", bufs=1))
                # prefetch D-phase weights during C (sync queue idle in C)
                wo_sb = pw.tile([D, HPC, E], F32R)
                nc.sync.dma_start(
                    out=wo_sb, in_=wop.ap().rearrange("p (c e) -> p c e", c=HPC)
                )
                bo_rep = pw.tile([D, E], F32)
                nc.scalar.dma_start(out=bo_rep, in_=bcast(bo, E))

                with ExitStack() as cc:
                    pc = cc.enter_context(tc.tile_pool(name="pc", bufs=2))
                    psc = cc.enter_context(
                        tc.tile_pool(name="psc", bufs=2, space="PSUM")
                    )

                    # One flat software pipeline over all (h, qt, tp) steps:
                    # attn·V + rowsum of step i issue after the scores of
                    # step i+1, crossing qt/head boundaries, so the PE never
                    # waits on exp.  PSUM double-buffering (bufs=2 on all
                    # tags) makes the cross-boundary overlap safe.
                    pending = None  # (h, qt, tp, ex, ps_ctx, ps_rs, g)

                    def flush_pending():
                        nonlocal pending
                        if pending is None:
                            return
                        h, qt, tp, ex, ps_ctx, ps_rs, g = pending
                        for j in range(2):
                            t_c = tp * 2 + j
                            nc.tensor.matmul(
                                ps_ctx,
                                vh[t_c][:, g * D : (g + 1) * D],
                                ex[:, j, :],
                                start=(t_c == 0),
                                stop=(t_c == NT - 1),
                            )
                            nc.tensor.matmul(
                                ps_rs,
                                ones_sb,
                                ex[:, j, :],
                                start=(t_c == 0),
                                stop=(t_c == NT - 1),
                            )
                        if tp == NT // 2 - 1:
                            # end of this (h, qt): normalize into cx
                            rr = pc.tile(
                                [D, 512], F32, name=f"rr{h}_{qt}", tag="rr", bufs=2
                            )
                            nc.vector.reciprocal(out=rr, in_=ps_rs)
                            nc.vector.tensor_mul(
                                out=cx[h][:, qt * 512 : (qt + 1) * 512],
                                in0=ps_ctx,
                                in1=rr,
                            )
                        pending = None

                    for h in range(HPC):
                        g = h // (HPC // GPC)
                        cx.append(qcx_tile(f"cx{h}"))
                        for qt in range(ST):
                            qsl = qh[h][:, qt * 512 : (qt + 1) * 512]
                            ps_ctx = psc.tile(
                                [D, 512], F32, name=f"ps_ctx{h}_{qt}", tag="ps_ctx"
                            )
                            ps_rs = psc.tile(
                                [D, 512], F32, name=f"ps_rs{h}_{qt}", tag="ps_rs"
                            )
                            for tp in range(NT // 2):
                                ps_sT = psc.tile(
                                    [D, 2, 512], F32, name=f"ps_sT{h}_{qt}_{tp}",
                                    tag="ps_sT",
                                )
                                for j in range(2):
                                    t_c = tp * 2 + j
                                    nc.tensor.matmul(
                                        ps_sT[:, j, :],
                                        kT[g][:, t_c * D : (t_c + 1) * D],
                                        qsl,
                                        start=True,
                                        stop=True,
                                    )
                                flush_pending()
                                ex = pc.tile(
                                    [D, 2, 512], F32R, name=f"ex{h}_{qt}_{tp}",
                                    tag="ex", bufs=3,
                                )
                                nc.scalar.activation(
                                    out=ex, in_=ps_sT, func=Exp, scale=SCALE
                                )
                                pending = (h, qt, tp, ex, ps_ctx, ps_rs, g)
                    flush_pending()

                # ---- Phase D: output projection + bias ----
                with ExitStack() as dd:
                    pd = dd.enter_context(tc.tile_pool(name="pd", bufs=2))
                    psd = dd.enter_context(
                        tc.tile_pool(name="psd", bufs=3, space="PSUM")
                    )
                    for et in range(ST):
                        for ss in range(NT):
                            ps_o = psd.tile(
                                [D, 512], F32, name=f"ps_o{et}_{ss}", tag="ps_o"
                            )
                            for hh in range(HPC):
                                nc.tensor.matmul(
                                    ps_o,
                                    cx[hh][:, ss * D : (ss + 1) * D],
                                    wo_sb[:, hh, et * 512 : (et + 1) * 512],
                                    start=(hh == 0),
                                    stop=(hh == HPC - 1),
                                )
                            ot = pd.tile(
                                [D, 512], F32, name=f"ot{et}_{ss}", tag="ot", bufs=3
                            )
                            nc.vector.tensor_add(
                                out=ot,
                                in0=ps_o,
                                in1=bo_rep[:, et * 512 : (et + 1) * 512],
                            )
                            nc.sync.dma_start(
                                out=out.ap()[
                                    ss * D : (ss + 1) * D, et * 512 : (et + 1) * 512
                                ],
                                in_=ot,
                            )

    nc.compile()
    return nc


def _get_program():
    global _PROGRAM
    if _PROGRAM is None:
        _PROGRAM = _build()
    return _PROGRAM


def make_in_maps(q, k, v, Wq, bq, Wk, bk, Wv, bv, Wo, bo):
    f32 = lambda a: np.asarray(a, dtype=np.float32)
    bf16 = lambda a: np.ascontiguousarray(a).astype(ml_dtypes.bfloat16)
    q, k, v = f32(q), f32(k), f32(v)
    Wq, bq, Wk, bk, Wv, bv, Wo, bo = (
        f32(Wq), f32(bq), f32(Wk), f32(bk), f32(Wv), f32(bv), f32(Wo), f32(bo)
    )
    in_maps = []
    for c in range(N_CORES):
        b, half = c // 2, c % 2
        wq_h = Wq[:, half * QC : (half + 1) * QC]      # [E, QC]
        wk_h = Wk[:, half * KV : (half + 1) * KV]      # [E, KV]
        wv_h = Wv[:, half * KV : (half + 1) * KV]
        wo_h = Wo[half * QC : (half + 1) * QC, :]      # [QC, E]
        # partition-major packs: row p holds every chunk's slice contiguously
        wkv_p = np.stack([wk_h.reshape(NE, D, KV), wv_h.reshape(NE, D, KV)], axis=2)
        wkv_p = wkv_p.transpose(1, 0, 2, 3).reshape(D, NE * 2 * KV)   # [p, n t c]
        wq_p = wq_h.reshape(NE, D, QC).transpose(1, 0, 2).reshape(D, NE * QC)
        wo_p = wo_h.reshape(HPC, D, E).transpose(1, 0, 2).reshape(D, HPC * E)
        bq_h = bq[half * QC : (half + 1) * QC].reshape(HPC, D).T      # [D, HPC]
        bk_h = bk[half * KV : (half + 1) * KV].reshape(GPC, D).T
        bv_h = bv[half * KV : (half + 1) * KV].reshape(GPC, D).T
        cstb = np.concatenate([bq_h, bk_h, bv_h], axis=1)
        cstm = np.concatenate(
            [np.ones((D, D), np.float32), np.eye(D, dtype=np.float32)], axis=1
        )
        in_maps.append(
            {
                "xq": bf16(q[b].T),
                "xk": bf16(k[b].T),
                "xv": bf16(v[b].T),
                "wkv": bf16(wkv_p),
                "wqp": bf16(wq_p),
                "wop": np.ascontiguousarray(wo_p),
                "cstb": np.ascontiguousarray(cstb),
                "cstm": np.ascontiguousarray(cstm),
                "bo": bo if half == 0 else np.zeros_like(bo),
            }
        )
    return in_maps


def combine_results(results):
    out = np.empty((B, S, E), np.float32)
    for b in range(B):
        out[b] = np.asarray(results[2 * b]["out_p"]) + np.asarray(
            results[2 * b + 1]["out_p"]
        )
    return out


def kernel(q, k, v, Wq, bq, Wk, bk, Wv, bv, Wo, bo):
    from concourse.bass_utils import run_bass_kernel_spmd

    nc = _get_program()
    in_maps = make_in_maps(q, k, v, Wq, bq, Wk, bk, Wv, bv, Wo, bo)
    res = run_bass_kernel_spmd(nc, in_maps, core_ids=list(range(N_CORES)))
    return combine_results(res.results)
